# revision 1
# baseline (speedup 1.0000x reference)
"""Trainium2 Bass kernel for AutoregressiveConvLSTM log-prob.

Strategy
--------
Data-parallel over batch: 64 images -> 8 NeuronCores, 8 images each.

Per-core layout: each (image-batch, channel) "plane" is an SBUF tile
[H=128 partitions, 1042 free] where image b occupies flat columns
130*b+1 .. 130*b+128 and the surrounding columns are zero pads.

All 3x3 convs run on the TensorEngine as banded matmuls:
  out[h_out, col] = sum_h_in Band[h_in, h_out] * plane[h_in, col+dx]
where Band is a [128,128] tri-diagonal matrix holding the three dy taps
(built on the host from the conv weights) and the dx in {-1,0,1} shift
is a free-dim AP offset into the zero pads.  Contributions over
(cin, dx) accumulate in PSUM.  Matmuls use float32r (full fp32 data,
fast PE mode).  LSTM pointwise math runs on ScalarE/VectorE in fp32.

The per-pixel log-prob terms are reduced over W on VectorE into a
[128 (=H), 8 (=image)] accumulator, and over H at the end with a single
ones-vector matmul.  Output per core: [8] -> host concatenates to [64].
"""

import numpy as np

B_FULL, C, H, W, F = 64, 16, 128, 128, 2
NCORES = 8
BL = B_FULL // NCORES            # images per core
WB = W + 2                       # per-image block width incl. pads
FREE = BL * WB + 2               # flat free size (+2 spare zero cols)
HALF_LOG_2PI = 0.9189385332046727

# chunks: (b0, n_imgs, c0, ilo)  — psum columns [c0, c0+n*WB), image b
# starts at local column WB*(b-b0)+ilo, interior slice [ilo, ilo+128)
CHUNKS = [(0, 3, 1, 0), (3, 3, 3 * WB, 1), (6, 2, 6 * WB, 1)]

N_STEP_BANDS = 3 + 72 + 12 + 12          # u, gates, head1, head2
N_ONETIME_BANDS = 96 + 12 + 12           # cond1, cond2, partial1
NB = N_ONETIME_BANDS + N_STEP_BANDS


def _band(w3):
    """[128,128] B[h_in,h_out] = w3[h_in-h_out+1] (tri-diagonal)."""
    b = np.zeros((H, H), np.float32)
    for dy in (-1, 0, 1):
        ar = np.arange(max(0, -dy), H - max(0, dy))
        b[ar + dy, ar] = w3[dy + 1]
    return b


def _build_bands(Wci, Wc1, Wc2, Wo1, Wo2, Wih, Whh):
    bands = np.zeros((NB, H, H), np.float32)
    i = 0
    # one-time: cond1 (16->2, ci-major for group streaming), cond2,
    # partial1 (cond_f part of Wo1)
    for ci in range(16):
        for co in range(2):
            for dx in range(3):
                bands[i] = _band(Wc1[:, dx, ci, co]); i += 1
    for co in range(2):
        for ci in range(2):
            for dx in range(3):
                bands[i] = _band(Wc2[:, dx, ci, co]); i += 1
    for co in range(2):
        for ci in range(2):
            for dx in range(3):
                bands[i] = _band(Wo1[:, dx, 2 + ci, co]); i += 1
    assert i == N_ONETIME_BANDS
    # step bands: u conv (1->1)
    for dx in range(3):
        bands[i] = _band(Wci[:, dx, 0, 0]); i += 1
    # gates: src 0,1 = h planes (Whh), src 2 = u plane (Wih)
    for co in range(8):
        for src in range(3):
            for dx in range(3):
                w3 = Whh[:, dx, src, co] if src < 2 else Wih[:, dx, 0, co]
                bands[i] = _band(w3); i += 1
    # head1 (h part of Wo1), head2 (Wo2)
    for co in range(2):
        for ci in range(2):
            for dx in range(3):
                bands[i] = _band(Wo1[:, dx, ci, co]); i += 1
    for co in range(2):
        for ci in range(2):
            for dx in range(3):
                bands[i] = _band(Wo2[:, dx, ci, co]); i += 1
    assert i == NB
    return bands


def _build_program(bci, bc1, bc2, bo1, bo2, bih):
    import concourse.bacc as bacc
    import concourse.mybir as mybir
    import concourse.tile as tile

    f32 = mybir.dt.float32
    MM = mybir.dt.float32r
    AF = mybir.ActivationFunctionType
    OP = mybir.AluOpType
    AX = mybir.AxisListType

    nc = bacc.Bacc("TRN2", target_bir_lowering=False, debug=False)
    xd = nc.dram_tensor("x", [BL, C, H, W], MM, kind="ExternalInput")
    cd = nc.dram_tensor("cond", [BL, C, H, W], MM, kind="ExternalInput")
    bd = nc.dram_tensor("bands", [NB, H, H], MM, kind="ExternalInput")
    od = nc.dram_tensor("out", [BL, 1], f32, kind="ExternalOutput")

    def i3(ap_flat, b0, n, lo):
        # [128, n, 128] interior view of a [128, >=1040] flat AP
        return ap_flat[:, : BL * WB].rearrange(
            "p (b w) -> p b w", w=WB)[:, b0:b0 + n, lo:lo + 128]

    with tile.TileContext(nc) as tc:
        import contextlib
        ctx = contextlib.ExitStack()
        with ctx:
            state = ctx.enter_context(tc.tile_pool(name="state", bufs=1))
            sbands = ctx.enter_context(tc.tile_pool(name="sbands", bufs=1))
            stream = ctx.enter_context(tc.tile_pool(name="stream", bufs=3))
            ctmp = ctx.enter_context(tc.tile_pool(name="ctmp", bufs=2))
            tmp = ctx.enter_context(tc.tile_pool(name="tmp", bufs=16))
            psum = ctx.enter_context(
                tc.tile_pool(name="psum", bufs=8, space="PSUM"))

            # step bands, resident
            sb = sbands.tile([H, N_STEP_BANDS * H], MM, tag="sb", name="sb")
            for k in range(8):
                s = (N_STEP_BANDS * k) // 8
                e = (N_STEP_BANDS * (k + 1)) // 8
                nc.sync.dma_start(
                    sb[:, s * H:e * H],
                    bd[N_ONETIME_BANDS + s:N_ONETIME_BANDS + e].rearrange(
                        "n p m -> p n m"))

            def band_st(i):
                return sb[:, i * H:(i + 1) * H].bitcast(MM)

            # persistent planes
            def plane(tag, dt=MM, memset=True):
                t = state.tile([H, FREE], dt, tag=tag)
                if memset:
                    nc.vector.memset(t[:].bitcast(f32), 0.0)
                return t

            h_pl = [plane("h0"), plane("h1")]
            c_pl = [plane("c0", f32), plane("c1", f32)]
            u_pl = plane("u")
            r_pl = [plane("r0"), plane("r1")]
            p1_pl = [plane("p1a", f32), plane("p1b", f32)]
            lp = state.tile([H, BL], f32, tag="lp", name="lp")
            nc.vector.memset(lp[:], 0.0)
            ones = state.tile([H, 1], f32, tag="ones", name="ones")
            nc.vector.memset(ones[:], 1.0)
            # bias columns: 0-7 bih, 8-9 bc1, 10-11 bc2, 12-13 bo1, 14 bci,
            # 15 = -bo2[1], 16 = final output bias
            cst = -16.0 * 128.0 * 128.0 * (float(bo2[1]) + HALF_LOG_2PI)
            bias_vals = (list(bih) + list(bc1) + list(bc2) + list(bo1)
                         + [float(bci[0]), -float(bo2[1]), cst])
            bias_t = state.tile([H, 17], f32, tag="bias", name="bias")
            for j, v in enumerate(bias_vals):
                nc.vector.memset(bias_t[:, j:j + 1], float(v))

            def bap(j, p=H):
                return bias_t[:p, j:j + 1]

            def new_plane(pool, src_dram, ci, tag, bufs=None):
                t = pool.tile([H, FREE], MM, tag=tag, name=tag, bufs=bufs)
                t3 = t[:, : BL * WB].rearrange("p (b w) -> p b w", w=WB)
                nc.vector.memset(t3[:, :, 0:1].bitcast(f32), 0.0)
                nc.vector.memset(t3[:, :, WB - 1:WB].bitcast(f32), 0.0)
                nc.vector.memset(t[:, BL * WB:].bitcast(f32), 0.0)
                nc.sync.dma_start(
                    t3[:, :, 1:129], src_dram[:, ci].rearrange("b h w -> h b w"))
                return t

            x_planes = {}

            def get_x(ci):
                if ci not in x_planes:
                    x_planes[ci] = new_plane(stream, xd, ci, "xpl")
                return x_planes[ci]

            # ---------------- cond phase ----------------
            with tc.tile_pool(name="otbands", bufs=2) as otp:
                GRP = 24
                ob_cur = [None]

                def load_group(g):
                    ob = otp.tile([H, GRP * H], MM, tag="ob", name="ob")
                    nc.sync.dma_start(
                        ob[:, :], bd[g * GRP:(g + 1) * GRP].rearrange(
                            "n p m -> p n m"))
                    ob_cur[0] = ob

                def band_ot(i):
                    j = i % GRP
                    return ob_cur[0][:, j * H:(j + 1) * H].bitcast(MM)

                # cond1: 16 -> 2, tanh
                pc = {}
                for co in range(2):
                    for k, (b0, n, c0, lo) in enumerate(CHUNKS):
                        pc[(co, k)] = psum.tile([H, 3 * WB], f32, tag="ps", name="ps")
                for ci in range(16):
                    if ci % 4 == 0:
                        load_group(ci // 4)
                    cpl = new_plane(stream, cd, ci, "cpl", bufs=2)
                    cf = cpl[:].bitcast(MM)
                    for co in range(2):
                        for k, (b0, n, c0, lo) in enumerate(CHUNKS):
                            for dx in (-1, 0, 1):
                                nc.tensor.matmul(
                                    pc[(co, k)][:, :n * WB],
                                    band_ot(ci * 6 + co * 3 + (dx + 1)),
                                    cf[:, c0 + dx:c0 + dx + n * WB],
                                    start=(ci == 0 and dx == -1),
                                    stop=(ci == 15 and dx == 1))
                tc_pl = [ctmp.tile([H, FREE], MM, tag="tc", name="tc") for _ in range(2)]
                for t in tc_pl:
                    nc.vector.memset(t[:].bitcast(f32), 0.0)
                for co in range(2):
                    for k, (b0, n, c0, lo) in enumerate(CHUNKS):
                        p3 = pc[(co, k)][:, :n * WB].rearrange(
                            "p (b w) -> p b w", w=WB)[:, :, lo:lo + 128]
                        nc.scalar.activation(
                            i3(tc_pl[co][:], b0, n, 1), p3, AF.Tanh,
                            bias=bap(8 + co))

                # cond2 -> cond_f planes; then partial1 = conv(cond_f)+bo1
                cf_pl = [ctmp.tile([H, FREE], MM, tag="cf", name="cf") for _ in range(2)]
                for t in cf_pl:
                    nc.vector.memset(t[:].bitcast(f32), 0.0)
                load_group(4)
                for dst, srcs, base, bias_col, out_pl in (
                        (cf_pl, tc_pl, 96, 10, None),
                        (None, cf_pl, 108, 12, p1_pl)):
                    tgt = dst if dst is not None else out_pl
                    for co in range(2):
                        for k, (b0, n, c0, lo) in enumerate(CHUNKS):
                            pq = psum.tile([H, 3 * WB], f32, tag="ps", name="ps")
                            first = True
                            for ci in range(2):
                                sf = srcs[ci][:].bitcast(MM)
                                for dx in (-1, 0, 1):
                                    nc.tensor.matmul(
                                        pq[:, :n * WB],
                                        band_ot(base + co * 6 + ci * 3 + dx + 1),
                                        sf[:, c0 + dx:c0 + dx + n * WB],
                                        start=first,
                                        stop=(ci == 1 and dx == 1))
                                    first = False
                            p3 = pq[:, :n * WB].rearrange(
                                "p (b w) -> p b w", w=WB)[:, :, lo:lo + 128]
                            nc.scalar.activation(
                                i3(tgt[co][:], b0, n, 1), p3, AF.Identity,
                                bias=bap(bias_col + co))

            # ---------------- steps ----------------
            def lp_tail(pq0, pq1, xt, b0, n, c0, lo):
                NN = n * WB
                e = tmp.tile([H, NN], f32, tag="tw", name="e")
                nc.scalar.activation(e[:], pq1[:, :NN], AF.Exp,
                                     bias=bap(15), scale=-1.0)
                d = tmp.tile([H, NN], f32, tag="tw", name="d")
                nc.vector.tensor_scalar(d[:], pq0[:, :NN], float(bo2[0]), None,
                                        OP.add)
                d2 = tmp.tile([H, NN], f32, tag="tw", name="d2")
                nc.vector.tensor_tensor(d2[:], xt[:, c0:c0 + NN].bitcast(f32), d[:],
                                        OP.subtract)
                z = tmp.tile([H, NN], f32, tag="tw", name="z")
                nc.vector.tensor_tensor(z[:], d2[:], e[:], OP.mult)
                s = tmp.tile([H, NN], f32, tag="tw", name="s")
                nc.scalar.activation(s[:], z[:], AF.Square,
                                     scale=0.7071067811865476)
                t = tmp.tile([H, NN], f32, tag="tw", name="t")
                nc.vector.tensor_tensor(t[:], s[:], pq1[:, :NN], OP.add)
                red = tmp.tile([H, n], f32, tag="tw", name="red")
                t3 = t[:].rearrange("p (b w) -> p b w", w=WB)[:, :, lo:lo + 128]
                nc.vector.reduce_sum(red[:], t3, AX.X)
                nc.vector.tensor_add(lp[:, b0:b0 + n], lp[:, b0:b0 + n], red[:])

            def head2_and_lp(xt_pl, b0, n, c0, lo):
                NN = n * WB
                pq = []
                for co in range(2):
                    q = psum.tile([H, 3 * WB], f32, tag="ps", name="ps")
                    first = True
                    for ci in range(2):
                        rf = r_pl[ci][:].bitcast(MM)
                        for dx in (-1, 0, 1):
                            nc.tensor.matmul(
                                q[:, :NN],
                                band_st(87 + co * 6 + ci * 3 + dx + 1),
                                rf[:, c0 + dx:c0 + dx + NN],
                                start=first, stop=(ci == 1 and dx == 1))
                            first = False
                    pq.append(q)
                lp_tail(pq[0], pq[1], xt_pl[:], b0, n, c0, lo)

            # step 0: feat = 0 -> r = relu(partial1)
            x0 = get_x(0)
            for (b0, n, c0, lo) in CHUNKS:
                for co in range(2):
                    nc.scalar.activation(
                        i3(r_pl[co][:], b0, n, 1),
                        i3(p1_pl[co][:], b0, n, 1), AF.Relu)
                head2_and_lp(x0, b0, n, c0, lo)

            for st in range(1, 16):
                xp = get_x(st - 1)
                xt = get_x(st)
                for (b0, n, c0, lo) in CHUNKS:
                    NN = n * WB
                    # u = conv(xp, Wci) + bci
                    pu = psum.tile([H, 3 * WB], f32, tag="ps", name="ps")
                    xf = xp[:].bitcast(MM)
                    for dx in (-1, 0, 1):
                        nc.tensor.matmul(pu[:, :NN], band_st(dx + 1),
                                         xf[:, c0 + dx:c0 + dx + NN],
                                         start=(dx == -1), stop=(dx == 1))
                    p3 = pu[:, :NN].rearrange(
                        "p (b w) -> p b w", w=WB)[:, :, lo:lo + 128]
                    nc.scalar.activation(i3(u_pl[:], b0, n, 1), p3,
                                         AF.Identity, bias=bap(14))
                    # gates
                    srcs = [h_pl[0], h_pl[1], u_pl]
                    pg = [None] * 8
                    for co in (0, 2, 4, 6, 1, 3, 5, 7):
                        g = psum.tile([H, 3 * WB], f32, tag="ps", name="ps")
                        first = True
                        for si, spl in enumerate(srcs):
                            sf = spl[:].bitcast(MM)
                            for dx in (-1, 0, 1):
                                nc.tensor.matmul(
                                    g[:, :NN],
                                    band_st(3 + co * 9 + si * 3 + dx + 1),
                                    sf[:, c0 + dx:c0 + dx + NN],
                                    start=first, stop=(si == 2 and dx == 1))
                                first = False
                        pg[co] = g
                    # LSTM pointwise (i,f,g,o = pg[0:2],[2:4],[4:6],[6:8])
                    for f in range(2):
                        ti = tmp.tile([H, NN], f32, tag="tw", name="ti")
                        nc.scalar.activation(ti[:], pg[f][:, :NN], AF.Sigmoid,
                                             bias=bap(f))
                        tg = tmp.tile([H, NN], f32, tag="tw", name="tg")
                        nc.scalar.activation(tg[:], pg[4 + f][:, :NN], AF.Tanh,
                                             bias=bap(4 + f))
                        tf = tmp.tile([H, NN], f32, tag="tw", name="tf")
                        nc.scalar.activation(tf[:], pg[2 + f][:, :NN],
                                             AF.Sigmoid, bias=bap(2 + f))
                        to = tmp.tile([H, NN], f32, tag="tw", name="to")
                        nc.scalar.activation(to[:], pg[6 + f][:, :NN],
                                             AF.Sigmoid, bias=bap(6 + f))
                        tig = tmp.tile([H, NN], f32, tag="tw", name="tig")
                        nc.vector.tensor_tensor(tig[:], ti[:], tg[:], OP.mult)
                        csl = c_pl[f][:, c0:c0 + NN]
                        nc.vector.tensor_tensor(csl, tf[:], csl, OP.mult)
                        nc.vector.tensor_tensor(csl, csl, tig[:], OP.add)
                        tc_ = tmp.tile([H, NN], f32, tag="tw", name="tc_")
                        nc.scalar.activation(tc_[:], csl, AF.Tanh)
                        to3 = to[:].rearrange(
                            "p (b w) -> p b w", w=WB)[:, :, lo:lo + 128]
                        tc3 = tc_[:].rearrange(
                            "p (b w) -> p b w", w=WB)[:, :, lo:lo + 128]
                        nc.vector.tensor_tensor(
                            i3(h_pl[f][:], b0, n, 1), to3, tc3, OP.mult)
                    # head1: r = relu(conv(h,Wo1[:, :, :2]) + partial1)
                    for co in range(2):
                        ph = psum.tile([H, 3 * WB], f32, tag="ps", name="ps")
                        first = True
                        for ci in range(2):
                            hf = h_pl[ci][:].bitcast(MM)
                            for dx in (-1, 0, 1):
                                nc.tensor.matmul(
                                    ph[:, :NN],
                                    band_st(75 + co * 6 + ci * 3 + dx + 1),
                                    hf[:, c0 + dx:c0 + dx + NN],
                                    start=first, stop=(ci == 1 and dx == 1))
                                first = False
                        hp = tmp.tile([H, NN], f32, tag="tw", name="hp")
                        nc.vector.tensor_tensor(
                            hp[:], ph[:, :NN], p1_pl[co][:, c0:c0 + NN], OP.add)
                        hp3 = hp[:].rearrange(
                            "p (b w) -> p b w", w=WB)[:, :, lo:lo + 128]
                        nc.scalar.activation(
                            i3(r_pl[co][:], b0, n, 1), hp3, AF.Relu)
                    head2_and_lp(xt, b0, n, c0, lo)

            # final: out = -(sum_p lp) - 16*128*128*(bo2[1] + HALF_LOG_2PI)
            po = psum.tile([BL, 1], f32, tag="ps", name="ps")
            nc.tensor.matmul(po[:], lp[:], ones[:], start=True, stop=True)
            osb = state.tile([BL, 1], f32, tag="osb", name="osb")
            nc.scalar.activation(osb[:], po[:], AF.Identity,
                                 scale=-1.0, bias=bap(16, BL))
            nc.sync.dma_start(od[:], osb[:])
    nc.compile()
    return nc


def kernel(**inputs):
    x = np.ascontiguousarray(inputs["x"], np.float32)
    cond = np.ascontiguousarray(inputs["cond"], np.float32)
    bands = _build_bands(
        np.asarray(inputs["Wci"], np.float32),
        np.asarray(inputs["Wc1"], np.float32),
        np.asarray(inputs["Wc2"], np.float32),
        np.asarray(inputs["Wo1"], np.float32),
        np.asarray(inputs["Wo2"], np.float32),
        np.asarray(inputs["Wih"], np.float32),
        np.asarray(inputs["Whh"], np.float32))
    nc = _build_program(
        np.asarray(inputs["bci"], np.float32),
        np.asarray(inputs["bc1"], np.float32),
        np.asarray(inputs["bc2"], np.float32),
        np.asarray(inputs["bo1"], np.float32),
        np.asarray(inputs["bo2"], np.float32),
        np.asarray(inputs["bih"], np.float32))
    from concourse.bass_utils import run_bass_kernel_spmd
    in_maps = [
        {"x": x[i * BL:(i + 1) * BL], "cond": cond[i * BL:(i + 1) * BL],
         "bands": bands}
        for i in range(NCORES)
    ]
    res = run_bass_kernel_spmd(nc, in_maps, list(range(NCORES)))
    out = np.concatenate(
        [res.results[i]["out"].reshape(BL) for i in range(NCORES)])
    return out.astype(np.float32)


if __name__ == "__main__":
    # smoke test with tiny random weights
    rng = np.random.default_rng(0)
    ins = {
        "x": rng.standard_normal((64, 16, 128, 128), np.float32),
        "cond": rng.standard_normal((64, 16, 128, 128), np.float32),
        "Wci": rng.standard_normal((3, 3, 1, 1), np.float32) * 0.1,
        "bci": np.zeros(1, np.float32),
        "Wc1": rng.standard_normal((3, 3, 16, 2), np.float32) * 0.1,
        "bc1": np.zeros(2, np.float32),
        "Wc2": rng.standard_normal((3, 3, 2, 2), np.float32) * 0.1,
        "bc2": np.zeros(2, np.float32),
        "Wo1": rng.standard_normal((3, 3, 4, 2), np.float32) * 0.1,
        "bo1": np.zeros(2, np.float32),
        "Wo2": rng.standard_normal((3, 3, 2, 2), np.float32) * 0.1,
        "bo2": np.zeros(2, np.float32),
        "Wih": rng.standard_normal((3, 3, 1, 8), np.float32) * 0.1,
        "bih": np.zeros(8, np.float32),
        "Whh": rng.standard_normal((3, 3, 2, 8), np.float32) * 0.1,
    }
    print(kernel(**ins)[:8])



# revision 6
# speedup vs baseline: 1.4383x; 1.4383x over previous
"""Trainium2 Bass kernel for AutoregressiveConvLSTM log-prob (v2).

Data-parallel over batch: 64 images -> 8 cores, 8 images each.

Per-core: planes are SBUF tiles [H=128 partitions, 1040 free] holding 8
image blocks of 130 cols ([pad][128][pad]).  All 3x3 convs run on
TensorE as banded matmuls in fp8e4m3 + DoubleRow: each pass applies two
[128,128] bands to two rhs plane views and accumulates both into PSUM
at 0.5 cycles/row.  Biases, the partial1 add and the (mean - x)
subtraction ride along as diagonal-band units against constant-ones /
x planes.  Sigmoid = 0.5*tanh(x/2)+0.5 so tanh/exp/relu/copy all live
in one act table (no table reloads).  PSUM = one [128,4096] arena, 32
slots of 128 f32 (4 per bank); gates use bank quarters 0-1, heads/u use
quarters 2-3, so the lag-2 software pipeline never collides.  LSTM
pointwise runs on VectorE in bf16 (2x/4x modes), GpSimd does the h
products + fp8 stores + z^2 reduction, and per-(img,step) log-prob
partials land in an accumulator via accum_out columns.
"""

import numpy as np
import ml_dtypes

B_FULL, C, H, W, F = 64, 16, 128, 128, 2
NCORES = 8
BL = B_FULL // NCORES
WB = W + 2
FREE = BL * WB                   # 1040
HALF_LOG_2PI = 0.9189385332046727
LNSQRT2 = 0.34657359027997264

X_ONES = 8                       # ones plane sits mid-arena

def _xslot(c):
    return c if c < 8 else c + 1

ST_H0, ST_H1, ST_U, ST_ONES = 0, 1, 2, 3

# gate psum planes in f-major emission order -> conv co
GATE_PLANES = [0, 2, 1, 3, 6, 7, 4, 5]   # i0 f0 i1 f1 o0 o1 g0 g1


def _band(w3):
    b = np.zeros((H, H), np.float32)
    for dy in (-1, 0, 1):
        ar = np.arange(max(0, -dy), H - max(0, dy))
        b[ar + dy, ar] = w3[dy + 1]
    return b


def _unit_band(u):
    kind = u[3]
    if kind == "w3":
        return _band(u[4])
    if kind == "diag":
        return np.eye(H, dtype=np.float32) * u[4]
    if kind == "zero":
        return np.zeros((H, H), np.float32)
    raise ValueError(kind)


def _build_tables(Wci, bci, Wc1, bc1, Wc2, bc2, Wo1, bo1, Wo2, bo2, Wih,
                  bih, Whh):
    """unit = (tile, slot, dx, kind, payload); returns pair tables."""
    bih_eff = bih + bci[0] * Wih.sum(axis=(0, 1, 2))

    def w3(Wt, dx, ci, co):
        return Wt[:, dx + 1, ci, co]

    u_fwd = []
    for dx, kind, val in ((-1, "diag", float(bci[0])), (0, "zero", 0.0),
                          (1, "zero", 0.0)):
        u_fwd.append((("xv", None, dx, "w3", Wci[:, dx + 1, 0, 0]),
                      ("x", X_ONES, 0, kind, val)))
    u_swp = [(b, a) for (a, b) in u_fwd]

    gates = []
    for co in GATE_PLANES:
        gates.append([
            (("st", ST_H0, -1, "w3", w3(Whh, -1, 0, co)),
             ("st", ST_H1, -1, "w3", w3(Whh, -1, 1, co))),
            (("st", ST_H0, 0, "w3", w3(Whh, 0, 0, co)),
             ("st", ST_H1, 0, "w3", w3(Whh, 0, 1, co))),
            (("st", ST_H0, 1, "w3", w3(Whh, 1, 0, co)),
             ("st", ST_U, -1, "w3", w3(Wih, -1, 0, co))),
            (("st", ST_H1, 1, "w3", w3(Whh, 1, 1, co)),
             ("st", ST_U, 0, "w3", w3(Wih, 0, 0, co))),
            (("st", ST_U, 1, "w3", w3(Wih, 1, 0, co)),
             ("st", ST_ONES, 0, "diag", float(bih_eff[co]))),
        ])

    head1 = []
    for co in range(2):
        prs = [(("st", ST_H0, dx, "w3", w3(Wo1, dx, 0, co)),
                ("st", ST_H1, dx, "w3", w3(Wo1, dx, 1, co)))
               for dx in (-1, 0, 1)]
        prs.append((("p1", 0, 0, "diag", 1.0 if co == 0 else 0.0),
                    ("p1", 1, 0, "diag", 1.0 if co == 1 else 0.0)))
        head1.append(prs)

    head2r = []
    for co in range(2):
        head2r.append([(("rr", 0, dx, "w3", w3(Wo2, dx, 0, co)),
                        ("rr", 1, dx, "w3", w3(Wo2, dx, 1, co)))
                       for dx in (-1, 0, 1)])
    head2x = []
    for co in range(2):
        fwd = (("xv", None, 0, "diag", -1.0 if co == 0 else 0.0),
               ("x", X_ONES, 0, "diag", float(bo2[co])))
        head2x.append(fwd)
        head2x.append((fwd[1], fwd[0]))

    cond1 = []
    for co in range(2):
        units = [("cond", _xslot(ci), dx, "w3", w3(Wc1, dx, ci, co))
                 for dx in (-1, 0, 1) for ci in range(16)]
        prs = [(units[2 * k], units[2 * k + 1]) for k in range(24)]
        prs.append((("cond", X_ONES, 0, "diag", float(bc1[co])),
                    ("cond", X_ONES + 1, 0, "zero", 0.0)))
        cond1.append(prs)

    cond2, part1 = [], []
    for co in range(2):
        cond2.append(
            [(("tc", 0, dx, "w3", w3(Wc2, dx, 0, co)),
              ("tc", 1, dx, "w3", w3(Wc2, dx, 1, co)))
             for dx in (-1, 0, 1)]
            + [(("tc", 0, 0, "zero", 0.0),
                ("tc", 2, 0, "diag", float(bc2[co])))])
        part1.append(
            [(("cf", 0, dx, "w3", w3(Wo1, dx, 2, co)),
              ("cf", 1, dx, "w3", w3(Wo1, dx, 3, co)))
             for dx in (-1, 0, 1)]
            + [(("cf", 0, 0, "zero", 0.0),
                ("cf", 2, 0, "diag", float(bo1[co])))])

    step = {"u_fwd": u_fwd, "u_swp": u_swp, "gates": gates,
            "head1": head1, "head2r": head2r, "head2x": head2x}
    onetime = {"cond1": cond1, "cond2": cond2, "part1": part1}
    return step, onetime


# flattened band order (indices shared by host array + device emitter)
def _flatten(step, onetime):
    flat = []
    flat += step["u_fwd"]                      # 0..2
    flat += step["u_swp"]                      # 3..5
    for prs in step["gates"]:                  # 6..45
        flat += prs
    for prs in step["head1"]:                  # 46..53
        flat += prs
    for prs in step["head2r"]:                 # 54..59
        flat += prs
    flat += step["head2x"]                     # 60..63
    oflat = []
    for key in ("cond1", "cond2", "part1"):
        for prs in onetime[key]:
            oflat += prs
    return flat, oflat

SP_UF, SP_US, SP_G, SP_H1, SP_H2R, SP_H2X = 0, 3, 6, 46, 54, 60
OT_C1, OT_C2, OT_P1 = 0, 50, 58


def _build_bands(tables):
    flat, oflat = _flatten(*tables)

    def arr(pairs):
        out = np.zeros((len(pairs), H, 2 * H), np.float32)
        for i, (ua, ub) in enumerate(pairs):
            out[i, :, :H] = _unit_band(ua)
            out[i, :, H:] = _unit_band(ub)
        # [N, K, 2M] -> partition-major [K, N*2M] for fast contiguous DMA
        return np.ascontiguousarray(
            out.transpose(1, 0, 2).reshape(H, -1)).astype(
                ml_dtypes.float8_e4m3fn)
    return arr(flat), arr(oflat), len(flat), len(oflat)


def build_nc(inputs):
    import contextlib

    import concourse.bacc as bacc
    import concourse.mybir as mybir
    import concourse.tile as tile
    from concourse.ap import AP

    tables = _build_tables(
        *[np.asarray(inputs[k], np.float32) for k in
          ("Wci", "bci", "Wc1", "bc1", "Wc2", "bc2", "Wo1", "bo1", "Wo2",
           "bo2", "Wih", "bih", "Whh")])
    step_t, onetime_t = tables
    flat, oflat = _flatten(step_t, onetime_t)
    NSP, NOT = len(flat), len(oflat)
    bo2 = np.asarray(inputs["bo2"], np.float32)

    f32 = mybir.dt.float32
    u32 = mybir.dt.uint32
    bf16 = mybir.dt.bfloat16
    fp8 = mybir.dt.float8e4
    AF = mybir.ActivationFunctionType
    OP = mybir.AluOpType
    DR = mybir.MatmulPerfMode.DoubleRow

    nc = bacc.Bacc("TRN2", target_bir_lowering=False, debug=False)
    xd = nc.dram_tensor("x", [C, H, BL * W], fp8, kind="ExternalInput")
    cd = nc.dram_tensor("cond", [C, H, BL * W], fp8, kind="ExternalInput")
    sbd = nc.dram_tensor("sbands", [H, NSP * 2 * H], fp8,
                         kind="ExternalInput")
    obd = nc.dram_tensor("obands", [H, NOT * 2 * H], fp8,
                         kind="ExternalInput")
    od = nc.dram_tensor("out", [BL, 1], f32, kind="ExternalOutput")

    with tile.TileContext(nc) as tc:
        ctx = contextlib.ExitStack()
        with ctx:
            big = ctx.enter_context(tc.tile_pool(name="big", bufs=1))
            ring = ctx.enter_context(tc.tile_pool(name="ring", bufs=4))
            pp = ctx.enter_context(
                tc.tile_pool(name="pp", bufs=1, space="PSUM"))

            xar = big.tile([H, 17 * FREE], fp8, tag="xar", name="xar")
            car = big.tile([H, 17 * FREE], fp8, tag="car", name="car")
            st = big.tile([H, 4 * FREE], fp8, tag="st", name="st")
            rr = big.tile([H, 2 * FREE], fp8, tag="rr", name="rr")
            p1p = big.tile([H, 2 * FREE], fp8, tag="p1p", name="p1p")
            tcar = big.tile([H, 3 * FREE], fp8, tag="tcar", name="tcar")
            cfar = big.tile([H, 3 * FREE], fp8, tag="cfar", name="cfar")
            cc = big.tile([H, 2 * FREE], bf16, tag="cc", name="cc")
            tcc = big.tile([H, 2 * FREE], bf16, tag="tcc", name="tcc")
            sbt = big.tile([H, NSP * 2 * H], fp8, tag="sbt", name="sbt")
            obt = big.tile([H, NOT * 2 * H], fp8, tag="obt", name="obt")
            rzl = big.tile([H, 2 * BL * C], f32, tag="rzl", name="rzl")
            lp = big.tile([H, BL], f32, tag="lp", name="lp")
            onef = big.tile([H, 1], f32, tag="onef", name="onef")
            fin = big.tile([BL, 2], f32, tag="fin", name="fin")
            ps = pp.tile([H, 4096], f32, tag="ps", name="ps")

            for t in (xar, car, st, rr, p1p, tcar, cfar, cc, tcc, rzl):
                nc.vector.memset(t[:].bitcast(u32), 0.0)
            for t, sl in ((xar, X_ONES), (car, X_ONES), (st, ST_ONES),
                          (tcar, 2), (cfar, 2)):
                nc.gpsimd.memset(t[:, sl * FREE:(sl + 1) * FREE], 1.0)
            nc.vector.memset(onef[:], 1.0)
            ebias = big.tile([H, 1], f32, tag="ebias", name="ebias")
            nc.vector.memset(ebias[:], -LNSQRT2)
            nc.vector.memset(lp[:], 0.0)

            nc.sync.dma_start(sbt[:], sbd[:])
            nc.sync.dma_start(obt[:], obd[:])

            def load_arena(arena, dram, ci):
                sl = _xslot(ci)
                dst = arena[:].rearrange(
                    "p (s b w) -> p s b w", s=17, w=WB)[
                        :, sl:sl + 1, :, 1:129]
                nc.sync.dma_start(
                    dst, dram[ci].rearrange("h (b w) -> h () b w", w=W))

            for ci in range(C):
                load_arena(xar, xd, ci)
                load_arena(car, cd, ci)

            # ---------- AP helpers ----------
            XT = {"x": xar, "xv": xar, "cond": car, "st": st, "rr": rr,
                  "p1": p1p, "tc": tcar, "cf": cfar}

            def rhs_pair(pair, img, xslot=None):
                ua, ub = pair
                t = XT[ua[0]]
                base = img * WB + 1

                def off(u):
                    sl = xslot if u[0] == "xv" else u[1]
                    return sl * FREE + base + u[2]

                oa, ob = off(ua), off(ub)
                assert 0 < ob - oa <= 15 * FREE, (pair, img)
                ext = t[:].ap[0][0]
                return AP(t[:].tensor, oa, [[ext, H], [ob - oa, 2], [1, W]])

            def lhs(bt, idx):
                return bt[:, idx * 2 * H:(idx + 1) * 2 * H].rearrange(
                    "p (s m) -> p s m", s=2)

            # psum: 32 slots of 128 words; slot S at word 128*S
            def pslot(S):
                return ps[:, 128 * S:128 * S + W]

            def pquad(bank0, nbank, q0, nq):
                return ps[:].rearrange(
                    "p (b q w) -> p b q w", q=4, w=W)[
                        :, bank0:bank0 + nbank, q0:q0 + nq, :]

            def interior(t, plane, img, n_img, npl=1):
                r = t[:].rearrange("p (pl b w) -> p pl b w", w=WB,
                                   pl=t.shape[1] // WB // BL)
                v = r[:, plane:plane + npl, img:img + n_img, 1:129]
                return v.squeeze(1) if npl == 1 else v

            def conv(S, items, img, xslot=None):
                """items: list of (band_idx, pair)."""
                out = pslot(S)
                n = len(items)
                for j, (bi, pr, bt) in enumerate(items):
                    nc.tensor.matmul(
                        out, lhs(bt, bi), rhs_pair(pr, img, xslot),
                        start=(j == 0), stop=(j == n - 1), perf_mode=DR)

            def sitems(base, pairs, bt):
                return [(base + k, pairs[k], bt) for k in range(len(pairs))]

            # ---------- ring tiles ----------
            def rt(name, wd, dt=bf16):
                return ring.tile([H, wd], dt, tag=name, name=name)

            # ---------- cond phase ----------
            for g in range(4):
                a = 2 * g
                for co in range(2):
                    for img in (a, a + 1):
                        conv(4 * (2 * g + co) + (img & 1),
                             sitems(OT_C1 + 25 * co,
                                    onetime_t["cond1"][co], obt), img)
            for g in range(4):
                a = 2 * g
                nc.scalar.activation(
                    interior(tcar, 0, a, 2, npl=2),
                    pquad(2 * g, 2, 0, 2), AF.Tanh)
                for co in range(2):
                    for img in (a, a + 1):
                        conv(4 * (2 * g + co) + (img & 1),
                             sitems(OT_C2 + 4 * co,
                                    onetime_t["cond2"][co], obt), img)
                nc.scalar.activation(
                    interior(cfar, 0, a, 2, npl=2),
                    pquad(2 * g, 2, 0, 2), AF.Copy)
                for co in range(2):
                    for img in (a, a + 1):
                        conv(4 * (2 * g + co) + (img & 1),
                             sitems(OT_P1 + 4 * co,
                                    onetime_t["part1"][co], obt), img)
                nc.scalar.activation(
                    interior(p1p, 0, a, 2, npl=2),
                    pquad(2 * g, 2, 0, 2), AF.Copy)

            nc.gpsimd.tensor_scalar_max(rr[:, :2 * FREE],
                                        p1p[:, :2 * FREE], 0.0)

            # ---------- step pipeline pieces ----------
            def two(t, base):
                return t[:, base:base + 2 * W].rearrange(
                    "p (i w) -> p i w", i=2)

            def emit_first(s, g):
                """PE u+gates for (s,g); Act ucopy; sigma/tanh; DVE c-chain;
                Pool h."""
                a = 2 * g
                uslot = _xslot(s - 1)
                ub, upairs = ((SP_UF, step_t["u_fwd"]) if uslot < X_ONES
                              else (SP_US, step_t["u_swp"]))
                for img in (a, a + 1):
                    conv(4 * 7 + 2 + (img & 1), sitems(ub, upairs, sbt),
                         img, xslot=uslot)
                nc.scalar.activation(
                    interior(st, ST_U, a, 2),
                    pquad(7, 1, 2, 2).squeeze(1), AF.Copy)
                for pl in range(8):
                    for img in (a, a + 1):
                        conv(4 * pl + (img & 1),
                             sitems(SP_G + 5 * pl,
                                    step_t["gates"][pl], sbt), img)
                gs = rt("gs", 6 * 2 * W)
                gt = rt("gt", 2 * 2 * W)
                nc.scalar.activation(
                    gs[:].rearrange("p (pl i w) -> p pl i w", pl=6, i=2),
                    pquad(0, 6, 0, 2), AF.Tanh, scale=0.5)
                nc.scalar.activation(
                    gt[:].rearrange("p (pl i w) -> p pl i w", pl=2, i=2),
                    pquad(6, 2, 0, 2), AF.Tanh)
                # sigma fix: sfx = 0.5*gs + 0.5 (blocks f0|f1|o)
                sfx = rt("sfx", 6 * 2 * W)
                for blk in range(3):
                    nc.vector.tensor_scalar(
                        sfx[:, blk * 4 * W:(blk + 1) * 4 * W],
                        gs[:, blk * 4 * W:(blk + 1) * 4 * W],
                        0.5, 0.5, OP.mult, OP.add)
                for f in range(2):
                    t1 = rt("t1", 2 * W)
                    c2 = rt("c2", 2 * W)
                    nc.vector.tensor_tensor(
                        t1[:], sfx[:, f * 4 * W:f * 4 * W + 2 * W],
                        gt[:, f * 2 * W:(f + 1) * 2 * W], OP.mult)
                    nc.vector.tensor_tensor(
                        two(c2, 0), two(sfx, f * 4 * W + 2 * W),
                        interior(cc, f, a, 2), OP.mult)
                    nc.vector.tensor_tensor(
                        interior(cc, f, a, 2), two(t1, 0), two(c2, 0),
                        OP.add)
                nc.scalar.activation(
                    interior(tcc, 0, a, 2, npl=2),
                    interior(cc, 0, a, 2, npl=2), AF.Tanh)
                for f in range(2):
                    hb = rt("hb", 2 * W)
                    nc.gpsimd.tensor_tensor(
                        two(hb, 0), two(sfx, 2 * 4 * W + f * 2 * W),
                        interior(tcc, f, a, 2), OP.mult)
                    nc.gpsimd.tensor_copy(
                        interior(st, ST_H0 + f, a, 2), two(hb, 0))

            def emit_h1(s, g):
                a = 2 * g
                for co in range(2):
                    for img in (a, a + 1):
                        conv(4 * co + 2 + (img & 1),
                             sitems(SP_H1 + 4 * co,
                                    step_t["head1"][co], sbt), img)

            def emit_h2(s, g, relu=True):
                a = 2 * g
                if relu:
                    nc.vector.tensor_scalar_max(
                        interior(rr, 0, a, 2, npl=2), pquad(0, 2, 2, 2),
                        0.0)
                xslot = _xslot(s)
                for co in range(2):
                    items = sitems(SP_H2R + 3 * co,
                                   step_t["head2r"][co], sbt)
                    xi = SP_H2X + 2 * co + (0 if xslot < X_ONES else 1)
                    items = items + [(xi, step_t["head2x"][xi - SP_H2X],
                                      sbt)]
                    for img in (a, a + 1):
                        conv(4 * (2 + co) + 2 + (img & 1), items, img,
                             xslot=xslot)
                ea = rt("ea", 2 * W)
                nc.scalar.activation(
                    two(ea, 0), pquad(3, 1, 2, 2).squeeze(1), AF.Exp,
                    bias=ebias[:], scale=-1.0)
                za = rt("za", 2 * W)
                nc.vector.tensor_tensor(
                    two(za, 0), pquad(2, 1, 2, 2).squeeze(1), two(ea, 0),
                    OP.mult)
                z2s = rt("z2s", 2 * W)
                lss = rt("lss", 2 * W)
                for j, img in enumerate((a, a + 1)):
                    col = img * C + s
                    zi = za[:, j * W:(j + 1) * W]
                    nc.vector.scalar_tensor_tensor(
                        z2s[:, j * W:(j + 1) * W], zi, 0.0, zi, OP.add,
                        OP.mult, accum_out=rzl[:, col:col + 1])
                    nc.vector.tensor_scalar(
                        lss[:, j * W:(j + 1) * W], pslot(4 * 3 + 2 + j),
                        1.0, None, OP.mult,
                        accum_out=rzl[:, BL * C + col:BL * C + col + 1])

            # ---------- schedule ----------
            # step 0: head2 only (r from relu(p1))
            sched = [(0, g, "h2only") for g in range(4)]
            for s in range(1, C):
                for g in range(4):
                    sched.append((s, g, "full"))

            pend = []          # (s, g) with h1/h2 outstanding
            for s, g, kind in sched:
                if kind == "h2only":
                    emit_h2(s, g, relu=False)
                    continue
                if pend and len(pend) >= 2:
                    emit_h1(*pend[0])
                emit_first(s, g)
                if pend and len(pend) >= 2:
                    emit_h2(*pend.pop(0))
                pend.append((s, g))
            for sg in pend:
                emit_h1(*sg)
                emit_h2(*sg)

            # ---------- finale ----------
            scr = big.tile([H, 2 * C], f32, tag="scr", name="scr")
            for b in range(BL):
                nc.vector.tensor_scalar(
                    scr[:].rearrange("p (h q) -> p h q", h=2),
                    rzl[:].rearrange(
                        "p (h q) -> p h q", h=2)[:, :, b * C:(b + 1) * C],
                    1.0, None, OP.mult,
                    accum_out=lp[:, b:b + 1])
            po = ps[0:BL, 0:1]
            nc.tensor.matmul(po, lp[:], onef[:], start=True, stop=True)
            cst = float(C * H * W * HALF_LOG_2PI)
            nc.scalar.activation(fin[:, 0:1], po, AF.Copy, scale=-1.0)
            nc.vector.tensor_scalar(fin[:, 1:2], fin[:, 0:1], -cst, None,
                                    OP.add)
            nc.sync.dma_start(od[:], fin[:, 1:2])
    nc.compile()
    return nc


def kernel(**inputs):
    x = np.ascontiguousarray(inputs["x"], np.float32)
    cond = np.ascontiguousarray(inputs["cond"], np.float32)
    tables = _build_tables(
        *[np.asarray(inputs[k], np.float32) for k in
          ("Wci", "bci", "Wc1", "bc1", "Wc2", "bc2", "Wo1", "bo1", "Wo2",
           "bo2", "Wih", "bih", "Whh")])
    sb, ob, _, _ = _build_bands(tables)
    nc = build_nc(inputs)

    fp8 = ml_dtypes.float8_e4m3fn
    in_maps = []
    for i in range(NCORES):
        xs = x[i * BL:(i + 1) * BL]          # [BL, C, H, W]
        cs = cond[i * BL:(i + 1) * BL]
        # -> [C, H, BL*W]
        xr = np.ascontiguousarray(
            xs.transpose(1, 2, 0, 3).reshape(C, H, BL * W)).astype(fp8)
        cr = np.ascontiguousarray(
            cs.transpose(1, 2, 0, 3).reshape(C, H, BL * W)).astype(fp8)
        in_maps.append({"x": xr, "cond": cr, "sbands": sb, "obands": ob})

    from concourse.bass_utils import run_bass_kernel_spmd
    res = run_bass_kernel_spmd(nc, in_maps, list(range(NCORES)))
    out = np.concatenate(
        [res.results[i]["out"].reshape(BL) for i in range(NCORES)])
    return out.astype(np.float32)


if __name__ == "__main__":
    rng = np.random.default_rng(0)
    ins = {
        "x": rng.standard_normal((64, 16, 128, 128)).astype(np.float32),
        "cond": rng.standard_normal((64, 16, 128, 128)).astype(np.float32),
        "Wci": (rng.standard_normal((3, 3, 1, 1)) * 0.1).astype(np.float32),
        "bci": np.zeros(1, np.float32),
        "Wc1": (rng.standard_normal((3, 3, 16, 2)) * 0.1).astype(np.float32),
        "bc1": np.zeros(2, np.float32),
        "Wc2": (rng.standard_normal((3, 3, 2, 2)) * 0.1).astype(np.float32),
        "bc2": np.zeros(2, np.float32),
        "Wo1": (rng.standard_normal((3, 3, 4, 2)) * 0.1).astype(np.float32),
        "bo1": np.zeros(2, np.float32),
        "Wo2": (rng.standard_normal((3, 3, 2, 2)) * 0.1).astype(np.float32),
        "bo2": np.zeros(2, np.float32),
        "Wih": (rng.standard_normal((3, 3, 1, 8)) * 0.1).astype(np.float32),
        "bih": np.zeros(8, np.float32),
        "Whh": (rng.standard_normal((3, 3, 2, 8)) * 0.1).astype(np.float32),
    }
    print(kernel(**ins)[:8])
